# revision 15
# baseline (speedup 1.0000x reference)
"""VQ codebook lookup (nn_AVQSeg) on 8 Trainium2 NeuronCores.

Data-parallel: x [8, 64, 64, 64] is sharded along batch (one batch element
-> 4096 tokens per core); the 8192x64 codebook is replicated.

Per core, the Bass kernel:
  1. scores s[n,k] = x_n . e_k - 0.5*||e_k||^2 via fp32 matmuls
     (argmin_k d ⟺ argmax_k s); the -0.5||e||^2 bias is computed on device
     and folded in as extra contraction rows. The codebook is fed in
     REVERSED order so that a last-position argmax = first-index argmin.
  2. argmax over k per token: a custom fused DVE op
     (select(x==runmax, Idx, -inf), accum=MAX) scans each PSUM quarter in
     ONE 1x pass, yielding 4 candidate indices/token/tile.
  3. candidates are re-scored exactly (d = ||x - e||^2, fp32) after an
     indirect-DMA gather; the winner (min d, first-index tie-break) is
     gathered into the outputs.
  4. emits x_q rows [4096, 64], x_q^T [64, 4096] (PE transposes), and
     x^T [4096, 64] (PE transposes of the input slice).
"""
import numpy as np

import concourse.bass as bass
import concourse.bacc as bacc
import concourse.tile as tile
import concourse.mybir as mybir
import concourse.dve_ops as dve_ops
from concourse.bass_utils import run_bass_kernel_spmd
from concourse.dve_spec import Spec, Src0, Idx, MaxNeg, scan, select, eq, AluOp, lower
from concourse.dve_uop import DveOpSpec
from concourse.dve_table_gen import dve_ver_for
from concourse.masks import make_identity

B, C, H, W = 8, 64, 64, 64
NTOK = H * W            # tokens per core = 4096
K = 8192                # codebook size
NTILE = NTOK // 128     # 32 token tiles of 128
NQ = 4                  # score quarters per tile (2048 codes each)
QW = K // NQ            # 2048
KC = C + 2              # contraction rows: 64 dims + bias hi/lo
f32 = mybir.dt.float32
f16 = mybir.dt.float16
u32 = mybir.dt.uint32
Alu = mybir.AluOpType
Ax = mybir.AxisListType

_CACHED = {}


def _register_last_argmax():
    """Fused DVE op: accum_out[p] = max{ k : in0[p,k] == runmax(in0[p,:k+1]) }
    i.e. the LAST argmax position along the free dim, in one 1x pass."""
    name = "LAST_ARGMAX_ANT"
    if name in dve_ops._SUB_OPCODE_FOR_NAME:
        return next(o for o in dve_ops.OPS if o.name == name)

    body = select(eq(Src0, scan(AluOp.MAX, Src0)), Idx, MaxNeg)

    def ref(in0):
        r = np.maximum.accumulate(in0, axis=1)
        n = in0.shape[1]
        idx = np.broadcast_to(np.arange(n, dtype=np.float32), in0.shape)
        out = np.where(in0 == r, idx, -np.finfo(np.float32).max)
        return out, out.max(axis=1, keepdims=True)

    spec = Spec(body=body, accum=AluOp.MAX, reference=ref)
    shas = {}
    for ver in ("v3", "v4"):
        try:
            uops = lower(spec, ver=ver)
            shas[ver] = DveOpSpec(name=name, opcode=0, uops=uops, rd1_en=False).sha(ver)
        except Exception:
            pass
    op = dve_ops.DveOp(name, spec, subdim=False, uops_sha=shas)
    dve_ops.OPS.append(op)
    dve_ops._SUB_OPCODE_FOR_NAME[name] = (
        dve_ops._CUSTOM_DVE_ROW_BASE + len(dve_ops.OPS) - 1
    )
    dve_ops.CUSTOM_DVE_SPECS[name] = spec
    return op


def build_nc(repeat: int = 1):
    import contextlib
    argmax_op = _register_last_argmax()
    nc = bacc.Bacc("TRN2", target_bir_lowering=False, debug=False)

    xb = nc.dram_tensor("xb", [C, NTOK], f32, kind="ExternalInput").ap()
    etp = nc.dram_tensor("etp", [C, K], f32, kind="ExternalInput").ap()  # reversed!
    erow = nc.dram_tensor("erow", [K, C], f32, kind="ExternalInput").ap()
    # fp16 hi/lo splits (host-prepared, same reversed order as etp)
    xhi_d = nc.dram_tensor("xhi", [C, NTOK], f16, kind="ExternalInput").ap()
    xlo_d = nc.dram_tensor("xlo", [C, NTOK], f16, kind="ExternalInput").ap()
    ehi_d = nc.dram_tensor("ehi", [C, K], f16, kind="ExternalInput").ap()
    elo_d = nc.dram_tensor("elo", [C, K], f16, kind="ExternalInput").ap()

    oq = nc.dram_tensor("oq", [NTOK, C], f32, kind="ExternalOutput").ap()
    oqd = nc.dram_tensor("oqd", [C, NTOK], f32, kind="ExternalOutput").ap()
    oxt = nc.dram_tensor("oxt", [NTOK, C], f32, kind="ExternalOutput").ap()

    with tile.TileContext(nc) as tc:
        with tc.tile_pool(name="persist", bufs=1) as pers:
            xs = pers.tile([C, NTOK], f32, tag="xs")
            xhi = pers.tile([KC, NTOK], f16, tag="xhi")
            xlo = pers.tile([KC, NTOK], f16, tag="xlo")
            ehi = pers.tile([KC, K], f16, tag="ehi")
            elo = pers.tile([KC, K], f16, tag="elo")
            qall = pers.tile([128, NTILE * C], f32, tag="qall")
            xtall = pers.tile([128, NTILE * C], f32, tag="xtall")
            qtall = pers.tile([C, NTOK], f32, tag="qtall")
            ident = pers.tile([128, 128], f32, tag="ident")
            ones = pers.tile([C, 1], f32, tag="ones")

            nc.sync.dma_start(xs[:], xb)
            nc.sync.dma_start(xhi[0:C, :], xhi_d)
            nc.sync.dma_start(xlo[0:C, :], xlo_d)
            nc.sync.dma_start(ehi[0:C, :], ehi_d)
            nc.sync.dma_start(elo[0:C, :], elo_d)
            # bias carrier rows: xhi rows 64,65 = 1; xlo rows = 0;
            # elo bias rows = 0; ehi bias rows filled below.
            nc.vector.memset(xhi[C : C + 2, :], 1.0)
            nc.vector.memset(xlo[C : C + 2, :], 0.0)
            nc.vector.memset(elo[C : C + 2, :], 0.0)
            make_identity(nc, ident[:])
            nc.vector.memset(ones[:], 1.0)

            rep_ctx = tc.For_i(0, repeat, 1) if repeat > 1 else \
                contextlib.nullcontext()
            rep_stack = contextlib.ExitStack()
            rep_stack.enter_context(rep_ctx)

            # ---- bias rows: es[64]+es[65] = -0.5 * ||e_k||^2 (hi + lo) ----
            with tc.tile_pool(name="biasps", bufs=1, space="PSUM") as bps, \
                 tc.tile_pool(name="biassb", bufs=2) as bsb:
                for h in range(2):  # two halves of 4096
                    sl = slice(h * 4096, (h + 1) * 4096)
                    esh = bsb.tile([C, 4096], f32, tag="esh")
                    nc.sync.dma_start(esh[:], etp[:, sl])
                    nc.scalar.square(esh[:], esh[:])
                    pb = bps.tile([1, 4096], f32, tag="pb")
                    for c in range(8):
                        nc.tensor.matmul(
                            pb[:, c * 512 : (c + 1) * 512],
                            ones[:],
                            esh[:, c * 512 : (c + 1) * 512],
                            start=True, stop=True,
                        )
                    # hi = fp16(-0.5*n); lo = fp16((-0.5*n) - hi)  (built at
                    # partition 0; compute engines need 32-aligned bases)
                    blo = bsb.tile([1, 4096], f32, tag="blo")
                    bh16 = bsb.tile([1, 4096], f16, tag="bh16")
                    bl16 = bsb.tile([1, 4096], f16, tag="bl16")
                    nc.scalar.mul(blo[:], pb[:], -0.5)
                    nc.vector.tensor_copy(bh16[:], blo[:])
                    nc.vector.tensor_sub(bl16[:], blo[:], bh16[:])
                    nc.sync.dma_start(ehi[C : C + 1, sl], bh16[:])
                    nc.sync.dma_start(ehi[C + 1 : C + 2, sl], bl16[:])

            # ---- pre-pass: x transposes -> oxt (also feeds the rescore) ----
            with tc.tile_pool(name="tps", bufs=4, space="PSUM") as tps:
                for t in range(NTILE):
                    pt = tps.tile([128, C], f32, tag="pt")
                    nc.tensor.transpose(
                        out=pt[:],
                        in_=xs[:, t * 128 : (t + 1) * 128],
                        identity=ident[0:C, 0:C],
                    )
                    xslot = xtall[:, t * C : (t + 1) * C]
                    nc.scalar.copy(xslot, pt[:])
                    nc.sync.dma_start(oxt[t * 128 : (t + 1) * 128, :], xslot)

            # ---- main loop: scores + fused argmax + rescore + gather ----
            qoff4 = pers.tile([128, NQ], f32, tag="qoff4")
            for q in range(NQ):
                # orig = 8191 - (q*2048 + k̂) = (-k̂) + (8191 - q*2048)
                nc.vector.memset(qoff4[:, q : q + 1], float(K - 1 - q * QW))

            with tc.tile_pool(name="scoreps", bufs=2, space="PSUM") as sps, \
                 tc.tile_pool(name="small", bufs=3) as ssb, \
                 tc.tile_pool(name="scratch", bufs=2) as scr:
                for t in range(NTILE):
                    hi = xhi[:, t * 128 : (t + 1) * 128]
                    lo = xlo[:, t * 128 : (t + 1) * 128]
                    xslot = xtall[:, t * C : (t + 1) * C]
                    kq4 = ssb.tile([128, NQ], f32, tag="kq4")
                    for q in range(NQ):
                        ps = sps.tile([128, QW], f32, tag="scores")
                        for c in range(QW // 512):
                            sl = slice(q * QW + c * 512, q * QW + (c + 1) * 512)
                            out = ps[:, c * 512 : (c + 1) * 512]
                            nc.tensor.matmul(out, hi, ehi[:, sl],
                                             start=True, stop=False)
                            nc.tensor.matmul(out, hi, elo[:, sl],
                                             start=False, stop=False)
                            nc.tensor.matmul(out, lo, ehi[:, sl],
                                             start=False, stop=True)
                        sc = scr.tile([128, QW], f16, tag="sc")
                        nc.vector._custom_dve(
                            argmax_op, out=sc[:], in0=ps[:],
                            accum_out=kq4[:, q : q + 1],
                        )
                    i4 = ssb.tile([128, NQ], f32, tag="i4")
                    nc.vector.tensor_sub(i4[:], qoff4[:], kq4[:])
                    ku4 = ssb.tile([128, NQ], u32, tag="ku4")
                    nc.vector.tensor_copy(ku4[:], i4[:])

                    # gather 4 candidate embedding rows
                    g4 = ssb.tile([128, NQ, C], f32, tag="g4")
                    for q in range(NQ):
                        nc.gpsimd.indirect_dma_start(
                            out=g4[:, q, :], out_offset=None, in_=erow,
                            in_offset=bass.IndirectOffsetOnAxis(
                                ap=ku4[:, q : q + 1], axis=0
                            ),
                        )

                    # exact d = sum((e - x)^2) per candidate
                    xbc = bass.AP(
                        xslot.tensor, xslot.offset,
                        [xslot.ap[0], [0, NQ], xslot.ap[1]],
                    )
                    diff = ssb.tile([128, NQ, C], f32, tag="diff")
                    nc.vector.tensor_sub(diff[:], g4[:], xbc)
                    nc.vector.tensor_mul(diff[:], diff[:], diff[:])
                    d4 = ssb.tile([128, NQ], f32, tag="d4")
                    nc.vector.tensor_reduce(d4[:], diff[:], axis=Ax.X, op=Alu.add)

                    # winner: min d, ties -> smallest original index
                    dmin = ssb.tile([128, 1], f32, tag="dmin")
                    nc.vector.tensor_reduce(dmin[:], d4[:], axis=Ax.X, op=Alu.min)
                    mask = ssb.tile([128, NQ], f32, tag="mask")
                    nc.vector.tensor_scalar(
                        mask[:], d4[:], dmin[:, 0:1], None, op0=Alu.is_le
                    )
                    nc.vector.tensor_scalar(
                        mask[:], mask[:], -1e9, 1e9, op0=Alu.mult, op1=Alu.add
                    )
                    nc.vector.tensor_add(mask[:], mask[:], i4[:])
                    kf = ssb.tile([128, 1], f32, tag="kf")
                    nc.vector.tensor_reduce(kf[:], mask[:], axis=Ax.X, op=Alu.min)
                    ku = ssb.tile([128, 1], u32, tag="ku")
                    nc.vector.tensor_copy(ku[:], kf[:])

                    qslot = qall[:, t * C : (t + 1) * C]
                    nc.gpsimd.indirect_dma_start(
                        out=qslot, out_offset=None, in_=erow,
                        in_offset=bass.IndirectOffsetOnAxis(ap=ku[:, :1], axis=0),
                    )
                    nc.sync.dma_start(oq[t * 128 : (t + 1) * 128, :], qslot)

            # ---- post-pass: quantized transposes -> oqd ----
            with tc.tile_pool(name="tps2", bufs=4, space="PSUM") as tps2:
                for t in range(NTILE):
                    pt = tps2.tile([C, 128], f32, tag="pt2")
                    nc.tensor.transpose(
                        out=pt[:],
                        in_=qall[:, t * C : (t + 1) * C],
                        identity=ident[:],
                    )
                    nc.scalar.copy(qtall[:, t * 128 : (t + 1) * 128], pt[:])
                nc.sync.dma_start(oqd, qtall[:])

            rep_stack.close()

    nc.compile()
    return nc


def kernel(x: np.ndarray, embedding: np.ndarray, _trace: bool = False):
    x = np.asarray(x, dtype=np.float32)
    embedding = np.asarray(embedding, dtype=np.float32)

    if "nc" not in _CACHED:
        _CACHED["nc"] = build_nc()
    nc = _CACHED["nc"]

    # codebook transposed AND reversed (so last-argmax == first-index argmin)
    etp = np.ascontiguousarray(embedding.T[:, ::-1])
    erow = np.ascontiguousarray(embedding)
    ehi = etp.astype(np.float16)
    elo = (etp - ehi.astype(np.float32)).astype(np.float16)
    in_maps = []
    for b in range(B):
        xbv = np.ascontiguousarray(x[b].reshape(C, NTOK))
        xhi = xbv.astype(np.float16)
        xlo = (xbv - xhi.astype(np.float32)).astype(np.float16)
        in_maps.append({
            "xb": xbv,
            "etp": etp,
            "erow": erow,
            "xhi": xhi,
            "xlo": xlo,
            "ehi": ehi,
            "elo": elo,
        })

    res = run_bass_kernel_spmd(nc, in_maps, core_ids=list(range(B)),
                               trace=_trace)
    _CACHED["last_results"] = res

    x_q = np.empty((B, H, W, C), dtype=np.float32)
    x_qd = np.empty((B, C, H, W), dtype=np.float32)
    x_bhwc = np.empty((B, H, W, C), dtype=np.float32)
    for b in range(B):
        out = res.results[b]
        x_q[b] = out["oq"].reshape(H, W, C)
        x_qd[b] = out["oqd"].reshape(C, H, W)
        x_bhwc[b] = out["oxt"].reshape(H, W, C)
    return (x_qd, x_q, x_bhwc)
